# revision 6
# baseline (speedup 1.0000x reference)
"""Trainium2 Bass kernel for nn_HGCLAMIR (HGCN contrastive model).

Full inputs in, full outputs out. Internally shards the 4096 graph nodes
row-wise across 8 NeuronCores. Per core (R=512 local rows):

  view v in {1,2}:
    t_v  = x_loc @ Wva + bva            [512,1024]   -> AllGather -> Tv [4096,1024]
    h1vT = prelu((G_loc @ Tv)^T, .25)   [1024,512]   (computed transposed)
    u_v  = h1v @ Wvb + bvb              [512,1024]   -> AllGather -> Uv
    h2vT = prelu((G_loc @ Uv)^T, .25)   [1024,512]
    zvT  = [h1vT; h2vT]                 [2048,512]   -> output (host transposes)
    proj chain (5 layers, elu) in transposed layout -> hpvT [256,512] f32
    nvT = hpvT / ||row||                -> AllGather -> NvT [2048,512]
  d12_r = n1_r . n2_r
  S sums via 4 similarity products vs gathered N blocks, exp row-sums on ACT
  l1_r = ln(S1+S12-e^2) - 2*d12_r ; l2_r = ln(S2+S21-e^2) - 2*d12_r

Host: z_v = gather(zvT)^T ; loss = 0.8*sum(l1) + 0.2*sum(l2).

All heavy matmuls use bf16 operands with fp32 PSUM accumulation.
"""

import numpy as np
import ml_dtypes

N = 4096
R = 512          # rows per core
H = 1024
P = 256          # projection dim
NCORES = 8
E2 = float(np.exp(2.0))  # exp(1/tau), tau=0.5: diag of refl for unit rows
ALPHA = 0.8

_BF = ml_dtypes.bfloat16

_cache = {}


def _build_program():
    from contextlib import ExitStack
    from concourse import mybir, tile, bacc

    dt = mybir.dt
    AF = mybir.ActivationFunctionType
    OP = mybir.AluOpType
    AX = mybir.AxisListType
    f32 = dt.float32
    bf = dt.bfloat16
    RG = [list(range(NCORES))]

    nc = bacc.Bacc("TRN2", target_bir_lowering=False, debug=False,
                   num_devices=NCORES)

    def I(name, shape, d=bf):
        return nc.dram_tensor(name, list(shape), d, kind="ExternalInput").ap()

    def O(name, shape, d):
        return nc.dram_tensor(name, list(shape), d, kind="ExternalOutput").ap()

    xT = [I("xT1", (N, R)), I("xT2", (N, R))]
    gT = [I("gT1", (N, R)), I("gT2", (N, R))]
    Wa = [I("W1a", (N, H)), I("W2a", (N, H))]
    Wb = [I("W1b", (H, H)), I("W2b", (H, H))]
    ba_d = [I("b1a", (128, H), f32), I("b2a", (128, H), f32)]
    bb_d = [I("b1b", (128, H), f32), I("b2b", (128, H), f32)]
    fc1w = I("fc1w", (2 * H, H))
    fc11w = I("fc11w", (H, 2 * P))
    fc12w = I("fc12w", (2 * P, P))
    fc2w = I("fc2w", (P, 2 * P))
    fc3w = I("fc3w", (2 * P, P))
    pb_d = [I("pb1", (H,), f32), I("pb11", (2 * P,), f32),
            I("pb12", (P,), f32), I("pb2", (2 * P,), f32),
            I("pb3", (P,), f32)]

    zT_o = [O("z1T", (2 * H, R), bf), O("z2T", (2 * H, R), bf)]
    l_o = [O("l1", (R,), f32), O("l2", (R,), f32)]

    with tile.TileContext(nc) as tc, ExitStack() as ctx:
        sb = ctx.enter_context(tc.tile_pool(name="sb", bufs=1))
        hpool = ctx.enter_context(tc.tile_pool(name="hpool", bufs=1))
        tout = ctx.enter_context(tc.tile_pool(name="tout", bufs=2))
        proj = ctx.enter_context(tc.tile_pool(name="proj", bufs=1))
        stream = ctx.enter_context(tc.tile_pool(name="stream", bufs=3))
        scr = ctx.enter_context(tc.tile_pool(name="scr", bufs=2))
        ps = ctx.enter_context(tc.tile_pool(name="ps", bufs=8, space="PSUM"))
        dram = ctx.enter_context(tc.tile_pool(name="dram", bufs=1,
                                              space="DRAM"))

        # ---- constants / biases in SBUF
        ones_col_f32 = sb.tile([128, 1], f32, name="ones_col_f32")
        nc.vector.memset(ones_col_f32[:], 1.0)
        ones_row_f32 = sb.tile([1, 128], f32, name="ones_row_f32")
        nc.vector.memset(ones_row_f32[:], 1.0)
        ones_col_bf = sb.tile([128, 1], bf, name="ones_col_bf")
        nc.vector.memset(ones_col_bf[:], 1.0)
        neg_e2 = sb.tile([128, 1], f32, name="neg_e2")
        nc.vector.memset(neg_e2[:], -E2)

        ba_sb, bb_sb = [], []
        for v in range(2):
            t_ = sb.tile([128, H], f32, name=f"ba{v}_sb")
            nc.sync.dma_start(t_[:], ba_d[v][:])
            ba_sb.append(t_)
            t_ = sb.tile([128, H], f32, name=f"bb{v}_sb")
            nc.sync.dma_start(t_[:], bb_d[v][:])
            bb_sb.append(t_)
        pb_sb = []
        for i, hh in enumerate([H, 2 * P, P, 2 * P, P]):
            t_ = sb.tile([128, hh // 128], f32, name=f"pb{i}_sb")
            nc.sync.dma_start(t_[:], pb_d[i].rearrange("(m p) -> p m", p=128))
            pb_sb.append(t_)

        # =====================================================
        def t_phase(v):
            """t = x_loc @ Wa + ba -> AllGather -> T [4096, 1024] bf16."""
            tps = [[ps.tile([128, 512], f32, tag="ps", name=f"t{v}ps{m}{n}")
                    for n in range(2)] for m in range(4)]
            xr = xT[v].rearrange("(k p) m -> p k m", p=128)
            wr = Wa[v].rearrange("(k p) h -> p k h", p=128)
            for k in range(32):
                xp = stream.tile([128, 512], bf, tag="xp", name="xp")
                nc.sync.dma_start(xp[:], xr[:, k, :])
                wp = stream.tile([128, 1024], bf, tag="wp", name="wp")
                nc.sync.dma_start(wp[:], wr[:, k, :])
                for m in range(4):
                    for n in range(2):
                        nc.tensor.matmul(
                            tps[m][n], xp[:, 128 * m:128 * (m + 1)],
                            wp[:, 512 * n:512 * (n + 1)],
                            start=(k == 0), stop=(k == 31))
            t_sb = tout.tile([128, 4, 1024], bf, tag="tout", name=f"t{v}_sb")
            for m in range(4):
                for n in range(2):
                    nc.vector.tensor_tensor(
                        t_sb[:, m, 512 * n:512 * (n + 1)], tps[m][n],
                        ba_sb[v][:, 512 * n:512 * (n + 1)], OP.add)
            ag_in = dram.tile([R, H], bf, name=f"ag_t{v}")
            nc.sync.dma_start(ag_in.rearrange("(mo p) h -> p mo h", p=128),
                              t_sb[:])
            T_g = dram.tile([N, H], bf, addr_space="Shared", name=f"Tg{v}")
            nc.gpsimd.collective_compute(
                "AllGather", OP.bypass, replica_groups=RG,
                ins=[ag_in.opt()], outs=[T_g.opt()])
            return T_g

        def h_phase(v, T_g, name, z_dram, row_off):
            """hT = prelu((G_loc @ T)^T) [1024, 512] bf16; also DMA to z."""
            hps = [ps.tile([128, 512], f32, tag="ps", name=f"{name}ps{m}")
                   for m in range(8)]
            Tr = T_g.rearrange("(k p) h -> p k h", p=128)
            Gr = gT[v].rearrange("(k p) r -> p k r", p=128)
            for k in range(32):
                tp = stream.tile([128, 1024], bf, tag="wp", name="tp")
                nc.sync.dma_start(tp[:], Tr[:, k, :])
                gp = stream.tile([128, 512], bf, tag="xp", name="gp")
                nc.sync.dma_start(gp[:], Gr[:, k, :])
                for m in range(8):
                    nc.tensor.matmul(hps[m], tp[:, 128 * m:128 * (m + 1)],
                                     gp[:], start=(k == 0), stop=(k == 31))
            h_sb = hpool.tile([128, 8, 512], bf, name=f"{name}_sb")
            for m in range(8):
                nc.scalar.activation(h_sb[:, m, :], hps[m], AF.Prelu,
                                     alpha=0.25)
                nc.sync.dma_start(
                    z_dram[row_off + 128 * m: row_off + 128 * (m + 1), :],
                    h_sb[:, m, :])
            return h_sb

        def u_phase(v, h_sb):
            """u = h1 @ Wb + bb -> AllGather -> U [4096, 1024] bf16."""
            ups = [[ps.tile([128, 512], f32, tag="ps", name=f"u{v}ps{m}{n}")
                    for n in range(2)] for m in range(4)]
            wr = Wb[v].rearrange("(k p) h -> p k h", p=128)
            for k in range(8):
                wp = stream.tile([128, 1024], bf, tag="wp", name="wbp")
                nc.sync.dma_start(wp[:], wr[:, k, :])
                for m in range(4):
                    for n in range(2):
                        nc.tensor.matmul(
                            ups[m][n], h_sb[:, k, 128 * m:128 * (m + 1)],
                            wp[:, 512 * n:512 * (n + 1)],
                            start=(k == 0), stop=(k == 7))
            u_sb = tout.tile([128, 4, 1024], bf, tag="tout", name=f"u{v}_sb")
            for m in range(4):
                for n in range(2):
                    nc.vector.tensor_tensor(
                        u_sb[:, m, 512 * n:512 * (n + 1)], ups[m][n],
                        bb_sb[v][:, 512 * n:512 * (n + 1)], OP.add)
            ag_in = dram.tile([R, H], bf, name=f"ag_u{v}")
            nc.sync.dma_start(ag_in.rearrange("(mo p) h -> p mo h", p=128),
                              u_sb[:])
            U_g = dram.tile([N, H], bf, addr_space="Shared", name=f"Ug{v}")
            nc.gpsimd.collective_compute(
                "AllGather", OP.bypass, replica_groups=RG,
                ins=[ag_in.opt()], outs=[U_g.opt()])
            return U_g

        def elu_drain(ps_t, bias_ap, out_ap):
            # elu(x+b) = min(exp(x+b)-1, relu(x+b))
            e = scr.tile([128, 512], f32, tag="elu_e", name="e_t")
            r = scr.tile([128, 512], f32, tag="elu_r", name="r_t")
            nc.scalar.activation(e[:], ps_t, AF.Exp, bias=bias_ap)
            nc.vector.tensor_scalar(r[:], ps_t, bias_ap, 0.0, OP.add, OP.max)
            nc.vector.tensor_scalar(e[:], e[:], -1.0, None, OP.add)
            nc.vector.tensor_tensor(out_ap, e[:], r[:], OP.min)

        def proj_layer(name, w_dram, k_sub, m_sub, rhs_fn, bias_sb, out_sb,
                       elu):
            pps = [ps.tile([128, 512], f32, tag="ps", name=f"{name}ps{m}")
                   for m in range(m_sub)]
            wr = w_dram.rearrange("(k p) h -> p k h", p=128)
            for k in range(k_sub):
                wp = stream.tile([128, m_sub * 128], bf, tag=f"fw{m_sub}",
                                 name="fwp")
                nc.sync.dma_start(wp[:], wr[:, k, :])
                r = rhs_fn(k)
                for m in range(m_sub):
                    nc.tensor.matmul(pps[m], wp[:, 128 * m:128 * (m + 1)], r,
                                     start=(k == 0), stop=(k == k_sub - 1))
            for m in range(m_sub):
                b_ap = bias_sb[:, m:m + 1]
                if elu:
                    elu_drain(pps[m], b_ap, out_sb[:, m, :])
                else:
                    nc.scalar.activation(out_sb[:, m, :], pps[m], AF.Identity,
                                         bias=b_ap)

        def proj_chain(v, h1_sb, h2_sb):
            p1 = proj.tile([128, 8, 512], bf, tag="p1", name=f"p1_{v}")
            proj_layer(f"pj1{v}", fc1w, 16, 8,
                       lambda k: h1_sb[:, k, :] if k < 8 else h2_sb[:, k - 8, :],
                       pb_sb[0], p1, True)
            p2 = proj.tile([128, 4, 512], bf, tag="p2", name=f"p2_{v}")
            proj_layer(f"pj2{v}", fc11w, 8, 4, lambda k: p1[:, k, :],
                       pb_sb[1], p2, True)
            p3 = proj.tile([128, 2, 512], bf, tag="p3", name=f"p3_{v}")
            proj_layer(f"pj3{v}", fc12w, 4, 2, lambda k: p2[:, k, :],
                       pb_sb[2], p3, True)
            p4 = proj.tile([128, 4, 512], bf, tag="p4", name=f"p4_{v}")
            proj_layer(f"pj4{v}", fc2w, 2, 4, lambda k: p3[:, k, :],
                       pb_sb[3], p4, True)
            hp = proj.tile([128, 2, 512], f32, tag="hp", name=f"hp_{v}")
            proj_layer(f"pj5{v}", fc3w, 4, 2, lambda k: p4[:, k, :],
                       pb_sb[4], hp, False)
            return hp

        def norm_phase(v, hp_sb):
            """nT = hpT / ||row|| (rows live on the free axis here)."""
            sq = scr.tile([128, 2, 512], f32, tag="sq", name=f"sq{v}")
            for kk in range(2):
                nc.scalar.activation(sq[:, kk, :], hp_sb[:, kk, :], AF.Square)
            nps = ps.tile([128, 512], f32, tag="ps", name=f"nsq{v}")
            for kk in range(2):
                nc.tensor.matmul(nps[:1, :], ones_col_f32[:], sq[:, kk, :],
                                 start=(kk == 0), stop=(kk == 1))
            nrm = scr.tile([1, 512], f32, tag="nrm", name=f"nrm{v}")
            nc.scalar.activation(nrm[:], nps[:1, :], AF.Sqrt)
            nc.vector.tensor_scalar(nrm[:], nrm[:], 1e-12, None, OP.max)
            inv = scr.tile([1, 512], f32, tag="inv", name=f"inv{v}")
            nc.vector.reciprocal(inv[:], nrm[:])
            bcp = ps.tile([128, 512], f32, tag="ps", name=f"bc{v}")
            nc.tensor.matmul(bcp[:], ones_row_f32[:], inv[:], start=True,
                             stop=True)
            nT = sb.tile([128, 2, 512], bf, name=f"n{v}T_sb")
            for kk in range(2):
                nc.vector.tensor_tensor(nT[:, kk, :], hp_sb[:, kk, :], bcp[:],
                                        OP.mult)
            ag_in = dram.tile([P, R], bf, name=f"ag_n{v}")
            nc.sync.dma_start(ag_in.rearrange("(ko p) r -> p ko r", p=128),
                              nT[:])
            NT_g = dram.tile([NCORES * P, R], bf, addr_space="Shared",
                             name=f"NTg{v}")
            nc.gpsimd.collective_compute(
                "AllGather", OP.bypass, replica_groups=RG,
                ins=[ag_in.opt()], outs=[NT_g.opt()])
            return nT, NT_g

        # ===================== pipeline =====================
        T1 = t_phase(0)
        T2 = t_phase(1)
        h11 = h_phase(0, T1, "h11", zT_o[0], 0)
        U1 = u_phase(0, h11)
        h12 = h_phase(1, T2, "h12", zT_o[1], 0)
        U2 = u_phase(1, h12)
        h21 = h_phase(0, U1, "h21", zT_o[0], H)
        h22 = h_phase(1, U2, "h22", zT_o[1], H)

        hp1 = proj_chain(0, h11, h21)
        n1T, N1g = norm_phase(0, hp1)
        hp2 = proj_chain(1, h12, h22)
        n2T, N2g = norm_phase(1, hp2)

        # ---- d2 = 2 * rowwise dot(n1, n2)  -> [128, 4] f32
        m12 = scr.tile([128, 2, 512], bf, tag="m12", name="m12")
        for kk in range(2):
            nc.vector.tensor_tensor(m12[:, kk, :], n1T[:, kk, :],
                                    n2T[:, kk, :], OP.mult)
        d2 = sb.tile([128, 4], f32, name="d2_sb")
        for m in range(4):
            dps = ps.tile([128, 512], f32, tag="ps", name=f"dps{m}")
            for kk in range(2):
                nc.tensor.matmul(dps[:, :1],
                                 m12[:, kk, 128 * m:128 * (m + 1)],
                                 ones_col_bf[:],
                                 start=(kk == 0), stop=(kk == 1))
            nc.vector.tensor_scalar(d2[:, m:m + 1], dps[:, :1], 2.0, None,
                                    OP.mult)

        # ---- similarity row sums + losses
        N1s = sb.tile([128, 16, 512], bf, name="N1s")
        nc.sync.dma_start(N1s[:], N1g.rearrange("(k p) r -> p k r", p=128))
        N2s = sb.tile([128, 16, 512], bf, name="N2s")
        nc.sync.dma_start(N2s[:], N2g.rearrange("(k p) r -> p k r", p=128))

        for m in range(4):
            acc1 = scr.tile([128, 16], f32, tag="acc", name=f"acc1_{m}")
            acc2 = scr.tile([128, 16], f32, tag="acc", name=f"acc2_{m}")
            for j in range(NCORES):
                prods = [(n1T, N1s, acc1, j), (n1T, N2s, acc1, 8 + j),
                         (n2T, N2s, acc2, j), (n2T, N1s, acc2, 8 + j)]
                for pi, (lt, rs, acc, col) in enumerate(prods):
                    sp = ps.tile([128, 512], f32, tag="ps",
                                 name=f"sp{m}_{j}_{pi}")
                    for kk in range(2):
                        nc.tensor.matmul(
                            sp[:], lt[:, kk, 128 * m:128 * (m + 1)],
                            rs[:, 2 * j + kk, :],
                            start=(kk == 0), stop=(kk == 1))
                    ex = scr.tile([128, 512], bf, tag="ex", name="ex")
                    nc.scalar.activation(ex[:], sp[:], AF.Exp, scale=2.0,
                                         accum_out=acc[:, col:col + 1])
            for acc, l_dram in [(acc1, l_o[0]), (acc2, l_o[1])]:
                ssum = scr.tile([128, 1], f32, tag="ssum", name=f"ssum{m}")
                nc.vector.reduce_sum(ssum[:], acc[:], axis=AX.X)
                ln_t = scr.tile([128, 1], f32, tag="lnt", name=f"lnt{m}")
                nc.scalar.activation(ln_t[:], ssum[:], AF.Ln,
                                     bias=neg_e2[:, :1])
                lt_ = scr.tile([128, 1], f32, tag="lout", name=f"lout{m}")
                nc.vector.tensor_tensor(lt_[:], ln_t[:], d2[:, m:m + 1],
                                        OP.subtract)
                nc.sync.dma_start(l_dram[128 * m:128 * (m + 1)], lt_[:])

    nc.finalize()
    return nc


def _get_program():
    if "nc" not in _cache:
        _cache["nc"] = _build_program()
    return _cache["nc"]


def kernel(**inputs):
    from concourse.bass_utils import run_bass_kernel_spmd

    f32 = np.float32
    x = [np.asarray(inputs["x1"], f32), np.asarray(inputs["x2"], f32)]
    adj = [np.asarray(inputs["adj1"], f32), np.asarray(inputs["adj2"], f32)]

    shared = {
        "W1a": np.asarray(inputs["W1a"], f32).astype(_BF),
        "W2a": np.asarray(inputs["W2a"], f32).astype(_BF),
        "W1b": np.asarray(inputs["W1b"], f32).astype(_BF),
        "W2b": np.asarray(inputs["W2b"], f32).astype(_BF),
        "b1a": np.broadcast_to(np.asarray(inputs["b1a"], f32),
                               (128, H)).astype(f32),
        "b2a": np.broadcast_to(np.asarray(inputs["b2a"], f32),
                               (128, H)).astype(f32),
        "b1b": np.broadcast_to(np.asarray(inputs["b1b"], f32),
                               (128, H)).astype(f32),
        "b2b": np.broadcast_to(np.asarray(inputs["b2b"], f32),
                               (128, H)).astype(f32),
        "fc1w": np.asarray(inputs["fc1_w"], f32).astype(_BF),
        "fc11w": np.asarray(inputs["fc11_w"], f32).astype(_BF),
        "fc12w": np.asarray(inputs["fc12_w"], f32).astype(_BF),
        "fc2w": np.asarray(inputs["fc2_w"], f32).astype(_BF),
        "fc3w": np.asarray(inputs["fc3_w"], f32).astype(_BF),
        "pb1": np.asarray(inputs["fc1_b"], f32),
        "pb11": np.asarray(inputs["fc11_b"], f32),
        "pb12": np.asarray(inputs["fc12_b"], f32),
        "pb2": np.asarray(inputs["fc2_b"], f32),
        "pb3": np.asarray(inputs["fc3_b"], f32),
    }

    in_maps = []
    for i in range(NCORES):
        rs = slice(R * i, R * (i + 1))
        m = dict(shared)
        m["xT1"] = x[0][rs].T.astype(_BF)
        m["xT2"] = x[1][rs].T.astype(_BF)
        m["gT1"] = adj[0][rs].T.astype(_BF)
        m["gT2"] = adj[1][rs].T.astype(_BF)
        in_maps.append(m)

    import time as _time

    nc = _get_program()
    t0 = _time.perf_counter()
    out = run_bass_kernel_spmd(nc, in_maps, list(range(NCORES)))
    t1 = _time.perf_counter()
    kernel._last_exec_ns = out.exec_time_ns
    if kernel._last_exec_ns is None:
        # no NTFF profiling in this container: wall time of the SPMD call
        # (compile+transfer+execute on first call; transfer+execute after)
        kernel._last_exec_ns = int((t1 - t0) * 1e9)
    res = out.results

    z1 = np.empty((N, 2 * H), f32)
    z2 = np.empty((N, 2 * H), f32)
    l1s = 0.0
    l2s = 0.0
    for i in range(NCORES):
        rs = slice(R * i, R * (i + 1))
        z1[rs] = res[i]["z1T"].astype(f32).T
        z2[rs] = res[i]["z2T"].astype(f32).T
        l1s += float(np.sum(res[i]["l1"].astype(np.float64)))
        l2s += float(np.sum(res[i]["l2"].astype(np.float64)))
    loss = np.float32(ALPHA * l1s + (1.0 - ALPHA) * l2s)
    return (z1, z2, loss)


kernel._last_exec_ns = None
